# revision 3
# baseline (speedup 1.0000x reference)
"""Trainium2 Bass kernel for the NFRNN z0-encoder.

Strategy: data-parallel over batch (4096 -> 8 cores x 512). Per core the
backward-time recurrence runs feature-major ([feat, batch]) with two
independent 256-wide batch chunks so PE/ACT/DVE/GPSIMD pipeline across the
sequential 64 steps. The whole pipeline is bf16 (weights, data slab, hidden
state, elementwise intermediates; PSUM accumulation stays fp32): matmuls run
at full PE rate and the bf16 SBUF-only DVE TensorTensor ops hit the 2x_1p
fast mode. Sigmoids are rewritten as tanh (keeps the loop on one ACT table
set with exp); the z-gate weights are host-negated so the gate tanh yields
tq = -tanh(gz/2) directly, giving the blend p = 0.5*m*(1+tq) and
qneg = (p-1)*x in one scalar_tensor_tensor each, and x <- p*n - qneg.
The n-path is tanh(0.5*(tr*hn + v)) where the PSUM region v = 2*inn + hn is
produced by running the whh_n matmul into two regions (wih_n host-prescaled
by 2), removing a DVE op from the chain. The gate tanh is split r/z so the
r half (on the critical path) retires one ACT slot earlier. Per-step dt
biases of the first flow matmul are folded in as K=1 rank-1 PE
accumulations (PSUM accumulation groups are kept strictly sequential per
bank - interleaving start/stop groups within a bank aborts at runtime).
Softplus of the std head and the final layout transpose happen on the host.
"""
import numpy as np
import ml_dtypes

import concourse.bass as bass
import concourse.mybir as mybir
import concourse.tile as tile
from concourse import bacc
from concourse.bass_utils import run_bass_kernel_spmd

B, T, IN_DIM = 4096, 64, 32
D = 2 * IN_DIM          # 64 data features
REC, HID, LAT = 128, 256, 64
NCORES = 8
BS = B // NCORES        # 512 batch per core
CH = BS // 2            # 256 batch per chunk
HT = T // 2             # data slab packs steps 0:32 on partitions 0:64, 32:64 on 64:128

F32 = mybir.dt.float32
F32R = mybir.dt.float32r
BF16 = mybir.dt.bfloat16
U8 = mybir.dt.uint8
AF = mybir.ActivationFunctionType
ALU = mybir.AluOpType

_CACHE = {}


def _bcast_ap(row_ap, parts):
    return bass.AP(tensor=row_ap.tensor, offset=row_ap.offset,
                   ap=[[0, parts]] + list(row_ap.ap)[1:])


def _build(flags, reps=1):
    (zb1_0, zb1_1, zb2_0, zb2_1, z_brz, z_bhhn) = flags
    nc = bacc.Bacc(enable_partition_id=False)

    dm = {}
    def din(name, shape, dt):
        dm[name] = nc.dram_tensor(name, shape, dt, kind="ExternalInput")
        return dm[name]

    dat_dm = din("dat", [128, HT * BS], BF16)
    msk_dm = din("msk", [T, BS], BF16)
    for l in range(2):
        din(f"w0aT{l}", [128, 256], BF16)
        din(f"w1T{l}", [128, 512], BF16)
        din(f"w2sT{l}", [128, 256], BF16)
        din(f"d0r{l}", [128, 4096], BF16)
        din(f"tsc{l}", [128, T], F32)
        din(f"tsh{l}", [128, T], F32)
        din(f"eb{l}", [128, T], F32)
        din(f"b1t{l}", [128, 2], F32)
        din(f"sb2t{l}", [128, T], F32)
    din("wihT", [128, 384], BF16)
    din("whhT", [128, 384], BF16)
    din("brzt", [128, 2], F32)
    din("biasn", [128, 1], F32)
    din("bhhn", [1, 128], BF16)
    din("ones", [128, 512], BF16)
    din("z0w0T", [128, 100], BF16)
    din("z0b0", [100, 1], F32)
    din("z0w1T", [100, 128], BF16)
    din("z0b1", [128, 1], F32)
    out_dm = nc.dram_tensor("out", [128, BS], F32, kind="ExternalOutput")

    with tile.TileContext(nc) as tc:
        with tc.tile_pool(name="const", bufs=1) as cp, \
             tc.tile_pool(name="shared", bufs=3) as shp, \
             tc.tile_pool(name="sb0", bufs=2) as sb0, \
             tc.tile_pool(name="sb1", bufs=2) as sb1, \
             tc.tile_pool(name="ps0", bufs=1, space="PSUM") as ps0, \
             tc.tile_pool(name="ps1", bufs=1, space="PSUM") as ps1:

            # ---- preload constants ----
            def load(name, shape, dt):
                t = cp.tile(shape, dt, tag=name)
                nc.sync.dma_start(out=t, in_=dm[name][tuple(slice(0, s) for s in shape)])
                return t

            dat = cp.tile([128, HT * BS], BF16, tag="dat")
            HB = HT * BS // 2
            for rh in range(2):
                for ch_ in range(2):
                    nc.sync.dma_start(
                        out=dat[rh * 64:(rh + 1) * 64, ch_ * HB:(ch_ + 1) * HB],
                        in_=dm["dat"][rh * 64:(rh + 1) * 64, ch_ * HB:(ch_ + 1) * HB])
            lay = []
            for l in range(2):
                lay.append(dict(
                    w0aT=load(f"w0aT{l}", [128, 256], BF16),
                    w1T=load(f"w1T{l}", [128, 512], BF16),
                    w2sT=load(f"w2sT{l}", [128, 256], BF16),
                    d0r=load(f"d0r{l}", [128, 4096], BF16),
                    tsc=load(f"tsc{l}", [128, T], F32),
                    tsh=load(f"tsh{l}", [128, T], F32),
                    eb=load(f"eb{l}", [128, T], F32),
                    b1t=load(f"b1t{l}", [128, 2], F32),
                    sb2t=load(f"sb2t{l}", [128, T], F32),
                ))
            wihT = load("wihT", [128, 384], BF16)
            whhT = load("whhT", [128, 384], BF16)
            brzt = load("brzt", [128, 2], F32)
            biasn = load("biasn", [128, 1], F32)
            bhhn = load("bhhn", [1, 128], BF16)
            ones = load("ones", [128, 512], BF16)
            z0w0T = load("z0w0T", [128, 100], BF16)
            z0b0 = load("z0b0", [100, 1], F32)
            z0w1T = load("z0w1T", [100, 128], BF16)
            z0b1 = load("z0b1", [128, 1], F32)
            out_sb = cp.tile([128, BS], F32, tag="out_sb")

            # persistent hidden state per chunk
            xs = []
            for c in range(2):
                x = cp.tile([128, CH], BF16, tag=f"x{c}")
                nc.vector.memzero(x[:, :])
                xs.append(x)

            zb1 = (zb1_0, zb1_1)
            zb2 = (zb2_0, zb2_1)
            sbp_ = (sb0, sb1)
            psp_ = (ps0, ps1)

            mreps = {}

            def get_mrep(s):
                if s not in mreps:
                    mrep = shp.tile([128, BS], BF16, tag="mrep")
                    nc.sync.dma_start(out=mrep[:, :],
                                      in_=_bcast_ap(msk_dm[s % T:s % T + 1, :], 128))
                    mreps[s] = mrep
                    if s - 2 in mreps:
                        del mreps[s - 2]
                return mreps[s]

            psum_t = {}

            def emit_flow(c, s_rep):
                    s = s_rep % T
                    sbp, psp, x = sbp_[c], psp_[c], xs[c]
                    part0 = (s // HT) * 64
                    col0 = (s % HT) * BS + c * CH
                    xt = dat[part0:part0 + 64, col0:col0 + CH]

                    T1 = psp.tile([128, 2 * CH], F32, tag="T1")
                    T2 = psp.tile([128, 2 * CH], F32, tag="T2", bufs=2)
                    T3 = psp.tile([128, CH], F32, tag="T3")

                    for l in range(2):
                        L = lay[l]
                        a0 = 0 if l == 0 else 64    # active (masked-in) dims
                        u0 = 64 - a0                # updated dims
                        rhs1 = x[a0:a0 + 64, :]
                        k0 = s * 2 + 0
                        r0, c0 = (k0 % 4) * 32, (k0 // 4) * 128
                        nc.tensor.matmul(T1[:, 0:CH], L["d0r"][r0:r0 + 1, c0:c0 + 128],
                                         ones[r0:r0 + 1, 0:CH], start=True, stop=False,
                                         tile_position=(r0, 0))
                        nc.tensor.matmul(T1[:, 0:CH], L["w0aT"][a0:a0 + 64, 0:128],
                                         rhs1, start=False, stop=True)
                        k1 = s * 2 + 1
                        r1, c1 = (k1 % 4) * 32, (k1 // 4) * 128
                        nc.tensor.matmul(T1[:, CH:2 * CH],
                                         L["d0r"][r1:r1 + 1, c1:c1 + 128],
                                         ones[r1:r1 + 1, 0:CH], start=True, stop=False,
                                         tile_position=(r1, 0))
                        nc.tensor.matmul(T1[:, CH:2 * CH], L["w0aT"][a0:a0 + 64, 128:256],
                                         rhs1, start=False, stop=True)
                        h1 = sbp.tile([128, 2 * CH], BF16, tag="h1")
                        nc.scalar.activation(h1[:, :], T1[:, :], AF.Tanh)
                        for mj in range(2):
                            for ki in range(2):
                                nc.tensor.matmul(
                                    T2[:, mj * CH:(mj + 1) * CH],
                                    L["w1T"][:, (2 * ki + mj) * 128:(2 * ki + mj) * 128 + 128],
                                    h1[:, ki * CH:(ki + 1) * CH],
                                    start=(ki == 0), stop=(ki == 1))
                        h2 = sbp.tile([128, 2 * CH], BF16, tag="h2")
                        if zb1[l]:
                            nc.scalar.activation(h2[:, :], T2[:, :], AF.Tanh)
                        else:
                            nc.scalar.activation(h2[:, 0:CH], T2[:, 0:CH], AF.Tanh,
                                                 bias=L["b1t"][:, 0:1])
                            nc.scalar.activation(h2[:, CH:2 * CH], T2[:, CH:2 * CH], AF.Tanh,
                                                 bias=L["b1t"][:, 1:2])
                        for ki in range(2):
                            nc.tensor.matmul(T3[:, :], L["w2sT"][:, ki * 128:(ki + 1) * 128],
                                             h2[:, ki * CH:(ki + 1) * CH],
                                             start=(ki == 0), stop=(ki == 1))
                        e = sbp.tile([128, CH], BF16, tag="e")
                        nc.scalar.activation(e[u0:u0 + 64, :], T3[0:64, :], AF.Exp,
                                             bias=L["eb"][u0:u0 + 64, s:s + 1],
                                             scale=L["tsc"][u0:u0 + 64, s:s + 1])
                        xe = sbp.tile([128, CH], BF16, tag="xe")
                        nc.vector.tensor_mul(xe[u0:u0 + 64, :], x[u0:u0 + 64, :],
                                             e[u0:u0 + 64, :])
                        nc.vector.scalar_tensor_tensor(x[u0:u0 + 64, :], T3[64:128, :],
                                                       L["tsh"][u0:u0 + 64, s:s + 1],
                                                       xe[u0:u0 + 64, :], ALU.mult, ALU.add)
                        if not zb2[l]:
                            nc.vector.tensor_scalar_add(x[u0:u0 + 64, :], x[u0:u0 + 64, :],
                                                        L["sb2t"][u0:u0 + 64, s:s + 1])

                    psum_t[c] = (T1, T2, T3, part0, col0)

            def emit_gru(c, s_rep):
                    s = s_rep % T
                    mrep = get_mrep(s_rep)
                    sbp, psp, x = sbp_[c], psp_[c], xs[c]
                    T1, T2, T3, part0, col0 = psum_t[c]
                    xt = dat[part0:part0 + 64, col0:col0 + CH]
                    # ---- GRU cell ----
                    wr = slice(part0, part0 + 64)
                    nc.tensor.matmul(T2[:, 0:CH], wihT[wr, 0:128], xt, start=True, stop=False)
                    nc.tensor.matmul(T2[:, 0:CH], whhT[:, 0:128], x[:, :], start=False, stop=True)
                    nc.tensor.matmul(T2[:, CH:2 * CH], wihT[wr, 128:256], xt, start=True, stop=False)
                    nc.tensor.matmul(T2[:, CH:2 * CH], whhT[:, 128:256], x[:, :], start=False, stop=True)
                    if z_bhhn:
                        nc.tensor.matmul(T1[:, CH:2 * CH], whhT[:, 256:384], x[:, :],
                                         start=True, stop=True)
                    else:
                        nc.tensor.matmul(T1[:, CH:2 * CH], whhT[:, 256:384], x[:, :],
                                         start=True, stop=False)
                        nc.tensor.matmul(T1[:, CH:2 * CH], bhhn[0:1, :], ones[0:1, 0:CH],
                                         start=False, stop=True)
                    nc.tensor.matmul(T1[:, 0:CH], wihT[wr, 256:384], xt, start=True, stop=False)
                    if not z_bhhn:
                        nc.tensor.matmul(T1[:, 0:CH], bhhn[0:1, :], ones[0:1, 0:CH],
                                         start=False, stop=False)
                    nc.tensor.matmul(T1[:, 0:CH], whhT[:, 256:384], x[:, :],
                                     start=False, stop=True)

                    trz = sbp.tile([128, 2 * CH], BF16, tag="trz")
                    if z_brz:
                        nc.scalar.activation(trz[:, 0:CH], T2[:, 0:CH], AF.Tanh, scale=0.5)
                        nc.scalar.activation(trz[:, CH:2 * CH], T2[:, CH:2 * CH],
                                             AF.Tanh, scale=0.5)
                    else:
                        nc.scalar.activation(trz[:, 0:CH], T2[:, 0:CH], AF.Tanh,
                                             scale=0.5, bias=brzt[:, 0:1])
                        nc.scalar.activation(trz[:, CH:2 * CH], T2[:, CH:2 * CH], AF.Tanh,
                                             scale=0.5, bias=brzt[:, 1:2])
                    tz = trz[:, CH:2 * CH]
                    wge = sbp.tile([128, CH], BF16, tag="wge")
                    nc.vector.tensor_mul(wge[:, :], trz[:, 0:CH], T1[:, CH:2 * CH])
                    npre = sbp.tile([128, CH], BF16, tag="npre")
                    nc.vector.tensor_add(npre[:, :], wge[:, :], T1[:, 0:CH])
                    ng = sbp.tile([128, CH], BF16, tag="ng")
                    nc.scalar.activation(ng[:, :], npre[:, :], AF.Tanh, scale=0.5,
                                         bias=biasn[:, 0:1])
                    # z weights negated on host: tz slot holds tq = tanh(-gz/2),
                    # p = 0.5*m*(1+tq), qneg = (p-1)*x, x_new = p*ng - qneg
                    p = sbp.tile([128, CH], BF16, tag="p")
                    nc.vector.scalar_tensor_tensor(p[:, :], tz, 1.0,
                                                   mrep[:, c * CH:(c + 1) * CH],
                                                   ALU.add, ALU.mult)
                    qneg = sbp.tile([128, CH], BF16, tag="qneg")
                    nc.vector.scalar_tensor_tensor(qneg[:, :], p[:, :], 1.0,
                                                   x[:, :], ALU.subtract, ALU.mult)
                    u1 = sbp.tile([128, CH], BF16, tag="u1")
                    nc.vector.tensor_mul(u1[:, :], p[:, :], ng[:, :])
                    nc.vector.tensor_tensor(x[:, :], u1[:, :], qneg[:, :],
                                            ALU.subtract)

            N = reps * T
            for s_rep in range(N + 1):
                if s_rep < N:
                    emit_flow(0, s_rep)
                if s_rep >= 1:
                    emit_gru(1, s_rep - 1)
                if s_rep < N:
                    emit_gru(0, s_rep)
                    emit_flow(1, s_rep)

            # ---- z0 head ----
            for c in range(2):
                sbp, psp, x = sbp_[c], psp_[c], xs[c]
                hd1 = psp.tile([100, CH], F32, tag="T3")
                nc.tensor.matmul(hd1[:, :], z0w0T[:, :], x[:, :], start=True, stop=True)
                hid = sbp.tile([100, CH], BF16, tag="h1")
                nc.scalar.activation(hid[:, :], hd1[:, :], AF.Tanh, bias=z0b0[:, 0:1])
                hd2 = psp.tile([128, CH], F32, tag="T1")
                nc.tensor.matmul(hd2[:, :], z0w1T[:, :], hid[:, :], start=True, stop=True)
                nc.scalar.activation(out_sb[:, c * CH:(c + 1) * CH], hd2[:, :],
                                     AF.Identity, bias=z0b1[:, 0:1])
            nc.sync.dma_start(out=out_dm[:, :], in_=out_sb[:, :])

    nc.compile()
    return nc


def _prep(inputs):
    f = {k: np.asarray(v, dtype=np.float32) for k, v in inputs.items()}
    data, ts = f["data"], f["time_steps"]
    dts = np.concatenate([np.float32([-0.01]), (ts[:-1] - ts[1:])[::-1]]).astype(np.float32)

    shared = {}
    flags = []
    for l in range(2):
        w0, b0 = f[f"f{l}_w0"], f[f"f{l}_b0"]
        w1, b1 = f[f"f{l}_w1"], f[f"f{l}_b1"]
        w2, b2 = f[f"f{l}_w2"], f[f"f{l}_b2"]
        tw = f[f"f{l}_tw"]
        act = slice(0, 64) if l == 0 else slice(64, 128)
        upd = slice(64, 128) if l == 0 else slice(0, 64)

        w0a = w0[:, act]                       # [256, 64]
        w0aT = np.ascontiguousarray(w0a.T)     # [64, 256]
        shared[f"w0aT{l}"] = np.concatenate([w0aT, w0aT], axis=0)  # [128,256] dup
        w1T = np.ascontiguousarray(w1.T)       # [256, 256] (in,out)
        blk = np.empty((128, 512), np.float32)
        for ki in range(2):
            for mj in range(2):
                blk[:, (2 * ki + mj) * 128:(2 * ki + mj) * 128 + 128] = \
                    w1T[ki * 128:(ki + 1) * 128, mj * 128:(mj + 1) * 128]
        shared[f"w1T{l}"] = blk
        ui = np.arange(128)[upd]
        w2s = np.concatenate([w2[ui, :], w2[128 + ui, :]], axis=0)  # [128, 256]
        b2s = np.concatenate([b2[ui], b2[128 + ui]])
        w2sT = np.ascontiguousarray(w2s.T)     # [256, 128]
        shared[f"w2sT{l}"] = np.concatenate([w2sT[0:128, :].reshape(128, 128),
                                             w2sT[128:256, :].reshape(128, 128)], axis=1)
        d0 = np.outer(w0[:, 128], dts) + b0[:, None]       # [256, T]
        d0r = np.zeros((128, 4096), np.float32)
        for s_ in range(T):
            for half in range(2):
                k = s_ * 2 + half
                d0r[(k % 4) * 32, (k // 4) * 128:(k // 4) * 128 + 128] = \
                    d0[half * 128:(half + 1) * 128, s_]
        shared[f"d0r{l}"] = d0r
        tt = np.tanh(np.outer(tw, dts))                    # [256, T]
        tscl = np.ascontiguousarray(tt[0:128][ui, :])    # [64, T]
        tshl = np.ascontiguousarray(tt[128:256][ui, :])  # [64, T]
        ebl = tscl * b2s[0:64, None]
        sb2l = tshl * b2s[64:128, None]
        dup = lambda v: np.ascontiguousarray(np.concatenate([v, v], axis=0))
        shared[f"tsc{l}"] = dup(tscl)
        shared[f"tsh{l}"] = dup(tshl)
        shared[f"eb{l}"] = dup(ebl)
        shared[f"sb2t{l}"] = dup(sb2l)
        b1t = np.stack([b1[0:128], b1[128:256]], axis=1)
        shared[f"b1t{l}"] = np.ascontiguousarray(b1t)
        flags += [bool(np.all(b1 == 0)), bool(np.all(b2 == 0))]

    wih, whh = f["gru_wih"], f["gru_whh"]
    bih, bhh = f["gru_bih"], f["gru_bhh"]
    wihT = np.ascontiguousarray(wih.T)                     # [64, 384]
    shared["wihT"] = np.concatenate([wihT, wihT], axis=0)  # [128, 384]
    shared["whhT"] = np.ascontiguousarray(whh.T)           # [128, 384]
    wihT = shared["wihT"].copy(); whhT = shared["whhT"].copy()
    wihT[:, 128:256] *= -1.0
    whhT[:, 128:256] *= -1.0
    wihT[:, 256:384] *= 2.0
    shared["wihT"] = wihT; shared["whhT"] = whhT
    brz = 0.5 * (bih[0:256] + bhh[0:256])
    brz = np.concatenate([brz[0:128], -brz[128:256]])
    shared["brzt"] = np.ascontiguousarray(np.stack([brz[0:128], brz[128:256]], axis=1))
    shared["biasn"] = np.ascontiguousarray(bih[256:384][:, None])
    shared["bhhn"] = np.ascontiguousarray(bhh[256:384][None, :])
    shared["ones"] = np.ones((128, 512), np.float32)
    shared["z0w0T"] = np.ascontiguousarray(f["z0_w0"].T)   # [128, 100]
    shared["z0b0"] = np.ascontiguousarray(f["z0_b0"][:, None])
    shared["z0w1T"] = np.ascontiguousarray(f["z0_w1"].T)   # [100, 128]
    shared["z0b1"] = np.ascontiguousarray(f["z0_b1"][:, None])
    flags += [bool(np.all(brz == 0)), bool(np.all(bhh[256:384] == 0))]
    # flags order: zb1_0, zb2_0, zb1_1, zb2_1, z_brz, z_bhhn -> reorder
    flags = (flags[0], flags[2], flags[1], flags[3], flags[4], flags[5])

    # data slab [D, T_rev, B] and masks
    arr = np.ascontiguousarray(data.transpose(2, 1, 0)[:, ::-1, :])   # [64, 64, 4096]
    mask = 0.5 * (data[:, ::-1, IN_DIM:].sum(axis=2) > 0).astype(np.float32)  # [B, T] rev
    mask = np.ascontiguousarray(mask.T)                               # [T, B]

    BF = ml_dtypes.bfloat16
    for k in ("wihT", "whhT", "z0w0T", "z0w1T", "bhhn", "ones"):
        shared[k] = shared[k].astype(BF)
    for l in range(2):
        for k in (f"w0aT{l}", f"w1T{l}", f"w2sT{l}", f"d0r{l}"):
            shared[k] = shared[k].astype(BF)

    in_maps = []
    for c in range(NCORES):
        sl = slice(c * BS, (c + 1) * BS)
        ac = arr[:, :, sl]                                            # [64, 64, 512]
        packed = np.empty((128, HT * BS), np.float32)
        packed[0:64, :] = ac[:, 0:HT, :].reshape(64, HT * BS)
        packed[64:128, :] = ac[:, HT:T, :].reshape(64, HT * BS)
        m = {"dat": packed.astype(BF), "msk": np.ascontiguousarray(mask[:, sl]).astype(BF)}
        m.update(shared)
        in_maps.append(m)
    return in_maps, flags


def kernel(**inputs):
    in_maps, flags = _prep(inputs)
    if _CACHE.get("flags") != flags:
        _CACHE["nc"] = _build(flags)
        _CACHE["flags"] = flags
    res = run_bass_kernel_spmd(_CACHE["nc"], in_maps, core_ids=list(range(NCORES)))
    _CACHE["last_res"] = res
    mean = np.empty((B, LAT), np.float32)
    stdp = np.empty((B, LAT), np.float32)
    for c in range(NCORES):
        o = res.results[c]["out"]                 # [128, 512]
        mean[c * BS:(c + 1) * BS] = o[0:LAT, :].T
        stdp[c * BS:(c + 1) * BS] = o[LAT:2 * LAT, :].T
    std = np.logaddexp(0.0, stdp).astype(np.float32)      # softplus
    return mean[None, :, :], std[None, :, :]



# revision 4
# speedup vs baseline: 1.5231x; 1.5231x over previous
"""Trainium2 Bass kernel for the NFRNN z0-encoder.

Strategy: data-parallel over batch (4096 -> 8 cores x 512). Per core the
backward-time recurrence runs feature-major ([feat, batch]) with two
independent 256-wide batch chunks so PE/ACT/DVE/GPSIMD pipeline across the
sequential 64 steps. The whole pipeline is bf16 (weights, data slab, hidden
state, elementwise intermediates; PSUM accumulation stays fp32): matmuls run
at full PE rate and the bf16 SBUF-only DVE TensorTensor ops hit the 2x_1p
fast mode. Sigmoids are rewritten as tanh (keeps the loop on one ACT table
set with exp); the z-gate weights are host-negated so the gate tanh yields
tq = -tanh(gz/2) directly, giving the blend p = 0.5*m*(1+tq) and
qneg = (p-1)*x in one scalar_tensor_tensor each, and x <- p*n - qneg.
The n-path is tanh(0.5*(tr*hn + v)) where the PSUM region v = 2*inn + hn is
produced by running the whh_n matmul into two regions (wih_n host-prescaled
by 2), removing a DVE op from the chain. The gate tanh is split r/z so the
r half (on the critical path) retires one ACT slot earlier. Per-step dt
biases of the first flow matmul are folded in as K=1 rank-1 PE
accumulations (PSUM accumulation groups are kept strictly sequential per
bank - interleaving start/stop groups within a bank aborts at runtime).
Softplus of the std head and the final layout transpose happen on the host.
"""
import numpy as np
import ml_dtypes

import concourse.bass as bass
import concourse.mybir as mybir
import concourse.tile as tile
from concourse import bacc
from concourse.bass_utils import run_bass_kernel_spmd

B, T, IN_DIM = 4096, 64, 32
D = 2 * IN_DIM          # 64 data features
REC, HID, LAT = 128, 256, 64
NCORES = 8
BS = B // NCORES        # 512 batch per core
CH = BS // 2            # 256 batch per chunk
HT = T // 2             # data slab packs steps 0:32 on partitions 0:64, 32:64 on 64:128

F32 = mybir.dt.float32
F32R = mybir.dt.float32r
BF16 = mybir.dt.bfloat16
U8 = mybir.dt.uint8
AF = mybir.ActivationFunctionType
ALU = mybir.AluOpType

_CACHE = {}


def _bcast_ap(row_ap, parts):
    return bass.AP(tensor=row_ap.tensor, offset=row_ap.offset,
                   ap=[[0, parts]] + list(row_ap.ap)[1:])


def _build(flags, reps=1):
    (zb1_0, zb1_1, zb2_0, zb2_1, z_brz, z_bhhn) = flags
    nc = bacc.Bacc(enable_partition_id=False)

    dm = {}
    def din(name, shape, dt):
        dm[name] = nc.dram_tensor(name, shape, dt, kind="ExternalInput")
        return dm[name]

    dat_dm = din("dat", [128, HT * BS], BF16)
    msk_dm = din("msk", [T, BS], BF16)
    for l in range(2):
        din(f"w0aT{l}", [128, 256], BF16)
        din(f"w1T{l}", [128, 512], BF16)
        din(f"w2sT{l}", [128, 256], BF16)
        din(f"d0r{l}", [128, 4096], BF16)
        din(f"tsc{l}", [128, T], F32)
        din(f"tsh{l}", [128, T], F32)
        din(f"eb{l}", [128, T], F32)
        din(f"b1t{l}", [128, 2], F32)
        din(f"sb2t{l}", [128, T], F32)
    din("wihT", [128, 384], BF16)
    din("whhT", [128, 384], BF16)
    din("brzt", [128, 2], F32)
    din("biasn", [128, 1], F32)
    din("bhhn", [1, 128], BF16)
    din("ones", [128, 512], BF16)
    din("z0w0T", [128, 100], BF16)
    din("z0b0", [100, 1], F32)
    din("z0w1T", [100, 128], BF16)
    din("z0b1", [128, 1], F32)
    out_dm = nc.dram_tensor("out", [128, BS], F32, kind="ExternalOutput")

    with tile.TileContext(nc) as tc:
        with tc.tile_pool(name="const", bufs=1) as cp, \
             tc.tile_pool(name="shared", bufs=3) as shp, \
             tc.tile_pool(name="sb0", bufs=2) as sb0, \
             tc.tile_pool(name="sb1", bufs=2) as sb1, \
             tc.tile_pool(name="ps0", bufs=1, space="PSUM") as ps0, \
             tc.tile_pool(name="ps1", bufs=1, space="PSUM") as ps1:

            # ---- preload constants ----
            def load(name, shape, dt):
                t = cp.tile(shape, dt, tag=name)
                nc.sync.dma_start(out=t, in_=dm[name][tuple(slice(0, s) for s in shape)])
                return t

            dat = cp.tile([128, HT * BS], BF16, tag="dat")
            HB = HT * BS // 2
            # first quarter of the data slab (time steps 0:16) loads before the
            # weights; the remaining 3MB queue after everything step-0 needs
            nc.sync.dma_start(out=dat[0:64, 0:HB], in_=dm["dat"][0:64, 0:HB])
            lay = []
            for l in range(2):
                lay.append(dict(
                    w0aT=load(f"w0aT{l}", [128, 256], BF16),
                    w1T=load(f"w1T{l}", [128, 512], BF16),
                    w2sT=load(f"w2sT{l}", [128, 256], BF16),
                    d0r=load(f"d0r{l}", [128, 4096], BF16),
                    tsc=load(f"tsc{l}", [128, T], F32),
                    tsh=load(f"tsh{l}", [128, T], F32),
                    eb=load(f"eb{l}", [128, T], F32),
                    b1t=load(f"b1t{l}", [128, 2], F32),
                    sb2t=load(f"sb2t{l}", [128, T], F32),
                ))
            wihT = load("wihT", [128, 384], BF16)
            whhT = load("whhT", [128, 384], BF16)
            brzt = load("brzt", [128, 2], F32)
            biasn = load("biasn", [128, 1], F32)
            bhhn = load("bhhn", [1, 128], BF16)
            ones = load("ones", [128, 512], BF16)
            z0w0T = load("z0w0T", [128, 100], BF16)
            z0b0 = load("z0b0", [100, 1], F32)
            z0w1T = load("z0w1T", [100, 128], BF16)
            z0b1 = load("z0b1", [128, 1], F32)
            nc.sync.dma_start(out=dat[0:64, HB:2 * HB], in_=dm["dat"][0:64, HB:2 * HB])
            nc.sync.dma_start(out=dat[64:128, 0:HB], in_=dm["dat"][64:128, 0:HB])
            nc.sync.dma_start(out=dat[64:128, HB:2 * HB],
                              in_=dm["dat"][64:128, HB:2 * HB])
            out_sb = cp.tile([128, BS], F32, tag="out_sb")

            # persistent hidden state per chunk
            xs = []
            for c in range(2):
                x = cp.tile([128, CH], BF16, tag=f"x{c}")
                nc.vector.memzero(x[:, :])
                xs.append(x)

            zb1 = (zb1_0, zb1_1)
            zb2 = (zb2_0, zb2_1)
            sbp_ = (sb0, sb1)
            psp_ = (ps0, ps1)

            mreps = {}

            def get_mrep(s):
                if s not in mreps:
                    mrep = shp.tile([128, BS], BF16, tag="mrep")
                    nc.sync.dma_start(out=mrep[:, :],
                                      in_=_bcast_ap(msk_dm[s % T:s % T + 1, :], 128))
                    mreps[s] = mrep
                    if s - 2 in mreps:
                        del mreps[s - 2]
                return mreps[s]

            psum_t = {}

            def emit_flow(c, s_rep):
                    s = s_rep % T
                    sbp, psp, x = sbp_[c], psp_[c], xs[c]
                    part0 = (s // HT) * 64
                    col0 = (s % HT) * BS + c * CH
                    xt = dat[part0:part0 + 64, col0:col0 + CH]

                    T1 = psp.tile([128, 2 * CH], F32, tag="T1")
                    T2 = psp.tile([128, 2 * CH], F32, tag="T2", bufs=2)
                    T3 = psp.tile([128, CH], F32, tag="T3")

                    for l in range(2):
                        L = lay[l]
                        a0 = 0 if l == 0 else 64    # active (masked-in) dims
                        u0 = 64 - a0                # updated dims
                        rhs1 = x[a0:a0 + 64, :]
                        k0 = s * 2 + 0
                        r0, c0 = (k0 % 4) * 32, (k0 // 4) * 128
                        nc.tensor.matmul(T1[:, 0:CH], L["d0r"][r0:r0 + 1, c0:c0 + 128],
                                         ones[r0:r0 + 1, 0:CH], start=True, stop=False,
                                         tile_position=(r0, 0))
                        nc.tensor.matmul(T1[:, 0:CH], L["w0aT"][a0:a0 + 64, 0:128],
                                         rhs1, start=False, stop=True)
                        k1 = s * 2 + 1
                        r1, c1 = (k1 % 4) * 32, (k1 // 4) * 128
                        nc.tensor.matmul(T1[:, CH:2 * CH],
                                         L["d0r"][r1:r1 + 1, c1:c1 + 128],
                                         ones[r1:r1 + 1, 0:CH], start=True, stop=False,
                                         tile_position=(r1, 0))
                        nc.tensor.matmul(T1[:, CH:2 * CH], L["w0aT"][a0:a0 + 64, 128:256],
                                         rhs1, start=False, stop=True)
                        h1 = sbp.tile([128, 2 * CH], BF16, tag="h1")
                        nc.scalar.activation(h1[:, :], T1[:, :], AF.Tanh)
                        for mj in range(2):
                            for ki in range(2):
                                nc.tensor.matmul(
                                    T2[:, mj * CH:(mj + 1) * CH],
                                    L["w1T"][:, (2 * ki + mj) * 128:(2 * ki + mj) * 128 + 128],
                                    h1[:, ki * CH:(ki + 1) * CH],
                                    start=(ki == 0), stop=(ki == 1))
                        h2 = sbp.tile([128, 2 * CH], BF16, tag="h2")
                        if zb1[l]:
                            nc.scalar.activation(h2[:, :], T2[:, :], AF.Tanh)
                        else:
                            nc.scalar.activation(h2[:, 0:CH], T2[:, 0:CH], AF.Tanh,
                                                 bias=L["b1t"][:, 0:1])
                            nc.scalar.activation(h2[:, CH:2 * CH], T2[:, CH:2 * CH], AF.Tanh,
                                                 bias=L["b1t"][:, 1:2])
                        for ki in range(2):
                            nc.tensor.matmul(T3[:, :], L["w2sT"][:, ki * 128:(ki + 1) * 128],
                                             h2[:, ki * CH:(ki + 1) * CH],
                                             start=(ki == 0), stop=(ki == 1))
                        e = sbp.tile([128, CH], BF16, tag="e")
                        nc.scalar.activation(e[u0:u0 + 64, :], T3[0:64, :], AF.Exp,
                                             bias=L["eb"][u0:u0 + 64, s:s + 1],
                                             scale=L["tsc"][u0:u0 + 64, s:s + 1])
                        xe = sbp.tile([128, CH], BF16, tag="xe")
                        nc.vector.tensor_mul(xe[u0:u0 + 64, :], x[u0:u0 + 64, :],
                                             e[u0:u0 + 64, :])
                        nc.vector.scalar_tensor_tensor(x[u0:u0 + 64, :], T3[64:128, :],
                                                       L["tsh"][u0:u0 + 64, s:s + 1],
                                                       xe[u0:u0 + 64, :], ALU.mult, ALU.add)
                        if not zb2[l]:
                            nc.vector.tensor_scalar_add(x[u0:u0 + 64, :], x[u0:u0 + 64, :],
                                                        L["sb2t"][u0:u0 + 64, s:s + 1])

                    psum_t[c] = (T1, T2, T3, part0, col0)

            def emit_gru(c, s_rep):
                    s = s_rep % T
                    mrep = get_mrep(s_rep)
                    sbp, psp, x = sbp_[c], psp_[c], xs[c]
                    T1, T2, T3, part0, col0 = psum_t[c]
                    xt = dat[part0:part0 + 64, col0:col0 + CH]
                    # ---- GRU cell ----
                    wr = slice(part0, part0 + 64)
                    nc.tensor.matmul(T2[:, 0:CH], wihT[wr, 0:128], xt, start=True, stop=False)
                    nc.tensor.matmul(T2[:, 0:CH], whhT[:, 0:128], x[:, :], start=False, stop=True)
                    nc.tensor.matmul(T2[:, CH:2 * CH], wihT[wr, 128:256], xt, start=True, stop=False)
                    nc.tensor.matmul(T2[:, CH:2 * CH], whhT[:, 128:256], x[:, :], start=False, stop=True)
                    if z_bhhn:
                        nc.tensor.matmul(T1[:, CH:2 * CH], whhT[:, 256:384], x[:, :],
                                         start=True, stop=True)
                    else:
                        nc.tensor.matmul(T1[:, CH:2 * CH], whhT[:, 256:384], x[:, :],
                                         start=True, stop=False)
                        nc.tensor.matmul(T1[:, CH:2 * CH], bhhn[0:1, :], ones[0:1, 0:CH],
                                         start=False, stop=True)
                    nc.tensor.matmul(T1[:, 0:CH], wihT[wr, 256:384], xt, start=True, stop=False)
                    if not z_bhhn:
                        nc.tensor.matmul(T1[:, 0:CH], bhhn[0:1, :], ones[0:1, 0:CH],
                                         start=False, stop=False)
                    nc.tensor.matmul(T1[:, 0:CH], whhT[:, 256:384], x[:, :],
                                     start=False, stop=True)

                    trz = sbp.tile([128, 2 * CH], BF16, tag="trz")
                    if z_brz:
                        nc.scalar.activation(trz[:, 0:CH], T2[:, 0:CH], AF.Tanh, scale=0.5)
                        nc.scalar.activation(trz[:, CH:2 * CH], T2[:, CH:2 * CH],
                                             AF.Tanh, scale=0.5)
                    else:
                        nc.scalar.activation(trz[:, 0:CH], T2[:, 0:CH], AF.Tanh,
                                             scale=0.5, bias=brzt[:, 0:1])
                        nc.scalar.activation(trz[:, CH:2 * CH], T2[:, CH:2 * CH], AF.Tanh,
                                             scale=0.5, bias=brzt[:, 1:2])
                    tz = trz[:, CH:2 * CH]
                    wge = sbp.tile([128, CH], BF16, tag="wge")
                    nc.vector.tensor_mul(wge[:, :], trz[:, 0:CH], T1[:, CH:2 * CH])
                    npre = sbp.tile([128, CH], BF16, tag="npre")
                    nc.vector.tensor_add(npre[:, :], wge[:, :], T1[:, 0:CH])
                    ng = sbp.tile([128, CH], BF16, tag="ng")
                    nc.scalar.activation(ng[:, :], npre[:, :], AF.Tanh, scale=0.5,
                                         bias=biasn[:, 0:1])
                    # z weights negated on host: tz slot holds tq = tanh(-gz/2),
                    # p = 0.5*m*(1+tq), qneg = (p-1)*x, x_new = p*ng - qneg
                    p = sbp.tile([128, CH], BF16, tag="p")
                    nc.vector.scalar_tensor_tensor(p[:, :], tz, 1.0,
                                                   mrep[:, c * CH:(c + 1) * CH],
                                                   ALU.add, ALU.mult)
                    qneg = sbp.tile([128, CH], BF16, tag="qneg")
                    nc.vector.scalar_tensor_tensor(qneg[:, :], p[:, :], 1.0,
                                                   x[:, :], ALU.subtract, ALU.mult)
                    u1 = sbp.tile([128, CH], BF16, tag="u1")
                    nc.vector.tensor_mul(u1[:, :], p[:, :], ng[:, :])
                    nc.vector.tensor_tensor(x[:, :], u1[:, :], qneg[:, :],
                                            ALU.subtract)

            N = reps * T
            for s_rep in range(N + 1):
                if s_rep < N:
                    emit_flow(0, s_rep)
                if s_rep >= 1:
                    emit_gru(1, s_rep - 1)
                if s_rep < N:
                    emit_gru(0, s_rep)
                    emit_flow(1, s_rep)

            # ---- z0 head ----
            for c in range(2):
                sbp, psp, x = sbp_[c], psp_[c], xs[c]
                hd1 = psp.tile([100, CH], F32, tag="T3")
                nc.tensor.matmul(hd1[:, :], z0w0T[:, :], x[:, :], start=True, stop=True)
                hid = sbp.tile([100, CH], BF16, tag="h1")
                nc.scalar.activation(hid[:, :], hd1[:, :], AF.Tanh, bias=z0b0[:, 0:1])
                hd2 = psp.tile([128, CH], F32, tag="T1")
                nc.tensor.matmul(hd2[:, :], z0w1T[:, :], hid[:, :], start=True, stop=True)
                nc.scalar.activation(out_sb[:, c * CH:(c + 1) * CH], hd2[:, :],
                                     AF.Identity, bias=z0b1[:, 0:1])
            nc.sync.dma_start(out=out_dm[:, :], in_=out_sb[:, :])

    nc.compile()
    return nc


def _prep(inputs):
    f = {k: np.asarray(v, dtype=np.float32) for k, v in inputs.items()}
    data, ts = f["data"], f["time_steps"]
    dts = np.concatenate([np.float32([-0.01]), (ts[:-1] - ts[1:])[::-1]]).astype(np.float32)

    shared = {}
    flags = []
    for l in range(2):
        w0, b0 = f[f"f{l}_w0"], f[f"f{l}_b0"]
        w1, b1 = f[f"f{l}_w1"], f[f"f{l}_b1"]
        w2, b2 = f[f"f{l}_w2"], f[f"f{l}_b2"]
        tw = f[f"f{l}_tw"]
        act = slice(0, 64) if l == 0 else slice(64, 128)
        upd = slice(64, 128) if l == 0 else slice(0, 64)

        w0a = w0[:, act]                       # [256, 64]
        w0aT = np.ascontiguousarray(w0a.T)     # [64, 256]
        shared[f"w0aT{l}"] = np.concatenate([w0aT, w0aT], axis=0)  # [128,256] dup
        w1T = np.ascontiguousarray(w1.T)       # [256, 256] (in,out)
        blk = np.empty((128, 512), np.float32)
        for ki in range(2):
            for mj in range(2):
                blk[:, (2 * ki + mj) * 128:(2 * ki + mj) * 128 + 128] = \
                    w1T[ki * 128:(ki + 1) * 128, mj * 128:(mj + 1) * 128]
        shared[f"w1T{l}"] = blk
        ui = np.arange(128)[upd]
        w2s = np.concatenate([w2[ui, :], w2[128 + ui, :]], axis=0)  # [128, 256]
        b2s = np.concatenate([b2[ui], b2[128 + ui]])
        w2sT = np.ascontiguousarray(w2s.T)     # [256, 128]
        shared[f"w2sT{l}"] = np.concatenate([w2sT[0:128, :].reshape(128, 128),
                                             w2sT[128:256, :].reshape(128, 128)], axis=1)
        d0 = np.outer(w0[:, 128], dts) + b0[:, None]       # [256, T]
        d0r = np.zeros((128, 4096), np.float32)
        for s_ in range(T):
            for half in range(2):
                k = s_ * 2 + half
                d0r[(k % 4) * 32, (k // 4) * 128:(k // 4) * 128 + 128] = \
                    d0[half * 128:(half + 1) * 128, s_]
        shared[f"d0r{l}"] = d0r
        tt = np.tanh(np.outer(tw, dts))                    # [256, T]
        tscl = np.ascontiguousarray(tt[0:128][ui, :])    # [64, T]
        tshl = np.ascontiguousarray(tt[128:256][ui, :])  # [64, T]
        ebl = tscl * b2s[0:64, None]
        sb2l = tshl * b2s[64:128, None]
        dup = lambda v: np.ascontiguousarray(np.concatenate([v, v], axis=0))
        shared[f"tsc{l}"] = dup(tscl)
        shared[f"tsh{l}"] = dup(tshl)
        shared[f"eb{l}"] = dup(ebl)
        shared[f"sb2t{l}"] = dup(sb2l)
        b1t = np.stack([b1[0:128], b1[128:256]], axis=1)
        shared[f"b1t{l}"] = np.ascontiguousarray(b1t)
        flags += [bool(np.all(b1 == 0)), bool(np.all(b2 == 0))]

    wih, whh = f["gru_wih"], f["gru_whh"]
    bih, bhh = f["gru_bih"], f["gru_bhh"]
    wihT = np.ascontiguousarray(wih.T)                     # [64, 384]
    shared["wihT"] = np.concatenate([wihT, wihT], axis=0)  # [128, 384]
    shared["whhT"] = np.ascontiguousarray(whh.T)           # [128, 384]
    wihT = shared["wihT"].copy(); whhT = shared["whhT"].copy()
    wihT[:, 128:256] *= -1.0
    whhT[:, 128:256] *= -1.0
    wihT[:, 256:384] *= 2.0
    shared["wihT"] = wihT; shared["whhT"] = whhT
    brz = 0.5 * (bih[0:256] + bhh[0:256])
    brz = np.concatenate([brz[0:128], -brz[128:256]])
    shared["brzt"] = np.ascontiguousarray(np.stack([brz[0:128], brz[128:256]], axis=1))
    shared["biasn"] = np.ascontiguousarray(bih[256:384][:, None])
    shared["bhhn"] = np.ascontiguousarray(bhh[256:384][None, :])
    shared["ones"] = np.ones((128, 512), np.float32)
    shared["z0w0T"] = np.ascontiguousarray(f["z0_w0"].T)   # [128, 100]
    shared["z0b0"] = np.ascontiguousarray(f["z0_b0"][:, None])
    shared["z0w1T"] = np.ascontiguousarray(f["z0_w1"].T)   # [100, 128]
    shared["z0b1"] = np.ascontiguousarray(f["z0_b1"][:, None])
    flags += [bool(np.all(brz == 0)), bool(np.all(bhh[256:384] == 0))]
    # flags order: zb1_0, zb2_0, zb1_1, zb2_1, z_brz, z_bhhn -> reorder
    flags = (flags[0], flags[2], flags[1], flags[3], flags[4], flags[5])

    # data slab [D, T_rev, B] and masks
    arr = np.ascontiguousarray(data.transpose(2, 1, 0)[:, ::-1, :])   # [64, 64, 4096]
    mask = 0.5 * (data[:, ::-1, IN_DIM:].sum(axis=2) > 0).astype(np.float32)  # [B, T] rev
    mask = np.ascontiguousarray(mask.T)                               # [T, B]

    BF = ml_dtypes.bfloat16
    for k in ("wihT", "whhT", "z0w0T", "z0w1T", "bhhn", "ones"):
        shared[k] = shared[k].astype(BF)
    for l in range(2):
        for k in (f"w0aT{l}", f"w1T{l}", f"w2sT{l}", f"d0r{l}"):
            shared[k] = shared[k].astype(BF)

    in_maps = []
    for c in range(NCORES):
        sl = slice(c * BS, (c + 1) * BS)
        ac = arr[:, :, sl]                                            # [64, 64, 512]
        packed = np.empty((128, HT * BS), np.float32)
        packed[0:64, :] = ac[:, 0:HT, :].reshape(64, HT * BS)
        packed[64:128, :] = ac[:, HT:T, :].reshape(64, HT * BS)
        m = {"dat": packed.astype(BF), "msk": np.ascontiguousarray(mask[:, sl]).astype(BF)}
        m.update(shared)
        in_maps.append(m)
    return in_maps, flags


def kernel(**inputs):
    in_maps, flags = _prep(inputs)
    if _CACHE.get("flags") != flags:
        _CACHE["nc"] = _build(flags)
        _CACHE["flags"] = flags
    res = run_bass_kernel_spmd(_CACHE["nc"], in_maps, core_ids=list(range(NCORES)))
    _CACHE["last_res"] = res
    mean = np.empty((B, LAT), np.float32)
    stdp = np.empty((B, LAT), np.float32)
    for c in range(NCORES):
        o = res.results[c]["out"]                 # [128, 512]
        mean[c * BS:(c + 1) * BS] = o[0:LAT, :].T
        stdp[c * BS:(c + 1) * BS] = o[LAT:2 * LAT, :].T
    std = np.logaddexp(0.0, stdp).astype(np.float32)      # softplus
    return mean[None, :, :], std[None, :, :]

